# revision 5
# baseline (speedup 1.0000x reference)
"""Two-layer GCN (PyG GCNConv x2 + rrelu) on 8 Trainium2 NeuronCores.

Math: with A = adjacency-with-multiplicity + I (self loops), deg = in-degree
(including the self loop), dinv = deg^-1/2:
    z1[v] = dinv[v] * (sum_{u->v} dinv[u]*x[u]) @ W1 + b1
    g[u]  = dinv[u] * rrelu(z1[u])                      (dinv pre-folded for L2)
    z2[v] = dinv[v] * (sum_{u->v} g[u]) @ W2 + b2

Sharding: destinations range-sharded across 8 cores (12544 each).  Every core
keeps a replicated (dinv-prescaled, bf16) source-feature table in HBM and
fetches the source rows of its edges with big dma_gather calls (one per
(superblock, source-window), ~4K indices).  Edge slots are packed
back-to-back with per-(block,window) segment lengths fixed to the max across
cores (SPMD uniformity).  Self-loop rows are staged host-side in a
partition-major layout and land in the message tile via one fat DMA per
superblock; their selector is the constant identity.  Scatter onto
destinations is a TensorE matmul with one-hot selectors generated on DVE (one
is_equal per destination block).  The epilogue runs in destination-major
orientation [dest, feat] so the per-destination dinv factors apply via the
ScalarE activation per-partition scale operand; outputs are written
node-major, which doubles as the next layer's source-table layout.
"""

import sys

for _p in ("/opt/trn_rl_repo",):
    if _p not in sys.path:
        sys.path.insert(0, _p)

import numpy as np
import ml_dtypes

import concourse.bacc as bacc
import concourse.bass as bass
import concourse.mybir as mybir
import concourse.tile as tile
from concourse.bass_utils import run_bass_kernel_spmd

P = 128
RRELU_SLOPE = (1.0 / 8.0 + 1.0 / 3.0) / 2.0


class Cfg:
    def __init__(self, n_nodes, n_cores, blocks_per_core, superblock, in_f,
                 out1_f, out2_f, src_window):
        self.n_nodes = n_nodes
        self.n_cores = n_cores
        self.bpc = blocks_per_core
        self.sb = superblock
        assert blocks_per_core % superblock == 0
        self.sb_count = blocks_per_core // superblock
        self.in_f = in_f
        self.out1_f = out1_f
        self.out2_f = out2_f
        self.src_window = src_window
        self.nodes_per_core = blocks_per_core * P
        self.n_pad = n_cores * self.nodes_per_core
        assert self.n_pad >= n_nodes
        self.n_chunks = -(-self.n_pad // src_window)
        self.tab_rows = self.n_chunks * src_window


FULL = Cfg(n_nodes=100000, n_cores=8, blocks_per_core=98, superblock=7,
           in_f=128, out1_f=128, out2_f=64, src_window=25088)


def _ru(x, m):
    return -(-x // m) * m


# --------------------------------------------------------------------------
# host-side index preprocessing
# --------------------------------------------------------------------------

def preprocess(edge_index, cfg):
    row = edge_index[0].astype(np.int64)
    col = edge_index[1].astype(np.int64)
    n = cfg.n_nodes
    npc = cfg.nodes_per_core
    NSB, NK, SBW = cfg.sb_count, cfg.n_chunks, cfg.sb

    deg = np.bincount(col, minlength=cfg.n_pad).astype(np.float64) + 1.0
    dinv = (1.0 / np.sqrt(deg)).astype(np.float32)
    dinv[n:] = 1.0

    core = col // npc
    col_loc = col % npc
    blk = col_loc >> 7
    s = blk // SBW
    b7 = blk % SBW
    k = row // cfg.src_window
    dloc = col_loc & 127

    cnt = np.zeros((cfg.n_cores, NSB, NK, SBW), dtype=np.int64)
    np.add.at(cnt, (core, s, k, b7), 1)
    seg_len = cnt.max(axis=0)             # [NSB, NK, SBW] uniform

    seg_start = np.zeros_like(seg_len)
    sec_pad = np.zeros((NSB, NK), dtype=np.int64)
    for si in range(NSB):
        for ki in range(NK):
            c0 = 0
            for b in range(SBW):
                seg_start[si, ki, b] = c0
                c0 += seg_len[si, ki, b]
            sec_pad[si, ki] = _ru(max(c0, 1), P)

    # msg tile columns: cols 0..6 = self rows; then per-k sections
    sec_col0 = np.zeros((NSB, NK), dtype=np.int64)
    msg_cols = np.zeros(NSB, dtype=np.int64)
    for si in range(NSB):
        c = SBW
        for ki in range(NK):
            sec_col0[si, ki] = c
            c += sec_pad[si, ki] // P
        msg_cols[si] = c

    idx_col0 = np.zeros((NSB, NK), dtype=np.int64)
    idx_cols = np.zeros(NSB, dtype=np.int64)
    for si in range(NSB):
        c = 0
        for ki in range(NK):
            idx_col0[si, ki] = c
            c += sec_pad[si, ki] // 16
        idx_cols[si] = c
    idx_off = np.concatenate([[0], np.cumsum(idx_cols)])
    ICOLS = int(idx_off[-1])

    # matmul schedule: per (s, b7) list of (msg_col, dcol) with dcol=-1 for
    # the identity (self) column
    mm_sched = [[None] * SBW for _ in range(NSB)]
    ndcols = np.zeros((NSB, SBW), dtype=np.int64)
    dcol_of = {}
    dcol_base = np.zeros((NSB, SBW), dtype=np.int64)
    DCOLS = 0
    for si in range(NSB):
        for b in range(SBW):
            ents = [(b, -1)]
            nd = 0
            for ki in range(NK):
                st = int(seg_start[si, ki, b])
                ln = int(seg_len[si, ki, b])
                if ln == 0:
                    continue
                c0, c1 = st // P, (st + ln - 1) // P
                for cc in range(c0, c1 + 1):
                    mcol = int(sec_col0[si, ki]) + cc
                    ents.append((mcol, nd))
                    dcol_of[(si, ki, cc, b)] = nd
                    nd += 1
            mm_sched[si][b] = ents
            ndcols[si, b] = nd
            dcol_base[si, b] = DCOLS
            DCOLS += nd
    G_MAX = int(ndcols.max())

    # per-edge slot assignment
    gid = ((core * NSB + s) * NK + k) * SBW + b7
    order = np.argsort(gid, kind="stable")
    gsort = gid[order]
    grp_start = np.zeros(cfg.n_cores * NSB * NK * SBW + 1, dtype=np.int64)
    np.cumsum(np.bincount(gsort, minlength=grp_start.size - 1), out=grp_start[1:])
    rank = np.empty(row.size, dtype=np.int64)
    rank[order] = np.arange(row.size) - grp_start[gsort]

    slot_in_sec = seg_start[s, k, b7] + rank
    sec_colv = slot_in_sec >> 7
    sec_p = slot_in_sec & 127
    idx_val = (row - k * cfg.src_window).astype(np.int16)
    idx_colv = idx_off[s] + idx_col0[s, k] + (slot_in_sec >> 4)
    idx_rowv = slot_in_sec & 15

    max_cols = int((sec_pad // P).max())
    dlk = np.full((NSB, NK, max_cols, SBW), -1, dtype=np.int64)
    for (si, ki, cc, b), v in dcol_of.items():
        dlk[si, ki, cc, b] = v
    dcol_l = dlk[s, k, sec_colv, b7]
    assert (dcol_l >= 0).all()
    d_colv = dcol_base[s, b7] + dcol_l

    per_core = []
    for c in range(cfg.n_cores):
        m = core == c
        it = np.zeros((16, ICOLS), dtype=np.int16)
        it[idx_rowv[m], idx_colv[m]] = idx_val[m]
        idx_tab = np.tile(it, (8, 1))
        d_tab = np.full((P, DCOLS), -1.0, dtype=np.float64)
        d_tab[sec_p[m], d_colv[m]] = dloc[m].astype(np.float64)
        dinv_pm = dinv[c * npc:(c + 1) * npc].reshape(cfg.bpc, P).T
        per_core.append({
            "idx_tab": np.ascontiguousarray(idx_tab),
            "d_tab": np.ascontiguousarray(d_tab.astype(ml_dtypes.bfloat16)),
            "dinv_pm": np.ascontiguousarray(dinv_pm),
        })

    shared = {
        "sec_pad": sec_pad, "sec_col0": sec_col0, "msg_cols": msg_cols,
        "idx_col0": idx_col0, "idx_cols": idx_cols, "idx_off": idx_off,
        "ICOLS": ICOLS, "DCOLS": DCOLS, "G_MAX": G_MAX,
        "mm_sched": mm_sched, "ndcols": ndcols, "dcol_base": dcol_base,
    }
    return {"dinv": dinv, "per_core": per_core, "shared": shared}


# --------------------------------------------------------------------------
# bass program (one GCN layer, SPMD across cores)
# --------------------------------------------------------------------------

def build_layer_program(cfg, shared, layer, max_call_idx=8064,
                        single_packet=False):
    NSB, NK, SBW = cfg.sb_count, cfg.n_chunks, cfg.sb
    out_f = cfg.out1_f if layer == 1 else cfg.out2_f
    out_dt = mybir.dt.bfloat16 if layer == 1 else mybir.dt.float32
    ICOLS, DCOLS, G_MAX = shared["ICOLS"], shared["DCOLS"], shared["G_MAX"]
    sec_pad, sec_col0 = shared["sec_pad"], shared["sec_col0"]
    msg_cols = shared["msg_cols"]
    idx_col0, idx_cols, idx_off = (shared["idx_col0"], shared["idx_cols"],
                                   shared["idx_off"])
    mm_sched, ndcols, dcol_base = (shared["mm_sched"], shared["ndcols"],
                                   shared["dcol_base"])
    MSG_MAX = int(msg_cols.max())
    IDX_MAX = int(idx_cols.max())

    nc = bacc.Bacc("TRN2", target_bir_lowering=False, debug=False,
                   num_devices=cfg.n_cores, num_swdge_queues=4)
    dt = mybir.dt
    src_tab = nc.dram_tensor("src_tab", [cfg.tab_rows, P], dt.bfloat16,
                             kind="ExternalInput")
    w_in = nc.dram_tensor("w", [P, out_f], dt.bfloat16, kind="ExternalInput")
    btile_in = nc.dram_tensor("btile", [P, out_f],
                              dt.bfloat16 if layer == 1 else dt.float32,
                              kind="ExternalInput")
    dinv_in = nc.dram_tensor("dinv_pm", [P, cfg.bpc], dt.float32,
                             kind="ExternalInput")
    idx_in = nc.dram_tensor("idx_tab", [P, ICOLS], dt.int16,
                            kind="ExternalInput")
    d_in = nc.dram_tensor("d_tab", [P, DCOLS], dt.bfloat16, kind="ExternalInput")
    iota_in = nc.dram_tensor("iota", [P, G_MAX * P], dt.bfloat16,
                             kind="ExternalInput")
    ident_in = nc.dram_tensor("ident", [P, P], dt.bfloat16, kind="ExternalInput")
    self_in = nc.dram_tensor("self_tab", [P, cfg.bpc * P], dt.bfloat16,
                             kind="ExternalInput")
    out_t = nc.dram_tensor("out_t", [cfg.nodes_per_core, out_f], out_dt,
                           kind="ExternalOutput")
    out_view = out_t.rearrange("(s b p) f -> s p b f", s=NSB, b=SBW, p=P)

    with tile.TileContext(nc) as tc:
        with (
            tc.tile_pool(name="const", bufs=1) as const_pool,
            tc.tile_pool(name="idx", bufs=2) as idx_pool,
            tc.tile_pool(name="msg", bufs=2) as msg_pool,
            tc.tile_pool(name="sel", bufs=3) as sel_pool,
            tc.tile_pool(name="aggsb", bufs=3) as aggsb_pool,
            tc.tile_pool(name="tmp", bufs=3) as tmp_pool,
            tc.tile_pool(name="outsb", bufs=2) as out_pool,
            tc.tile_pool(name="psA", bufs=2, space="PSUM") as agg_psum,
            tc.tile_pool(name="psZ", bufs=2, space="PSUM") as z_psum,
        ):
            w_sb = const_pool.tile([P, out_f], dt.bfloat16)
            nc.sync.dma_start(out=w_sb[:], in_=w_in[:])
            btile_sb = const_pool.tile([P, out_f],
                                       dt.bfloat16 if layer == 1 else dt.float32)
            nc.sync.dma_start(out=btile_sb[:], in_=btile_in[:])
            dinv_sb = const_pool.tile([P, cfg.bpc], dt.float32)
            nc.sync.dma_start(out=dinv_sb[:], in_=dinv_in[:])
            iota_sb = const_pool.tile([P, G_MAX * P], dt.bfloat16)
            nc.sync.dma_start(out=iota_sb[:], in_=iota_in[:])
            ident_sb = const_pool.tile([P, P], dt.bfloat16)
            nc.sync.dma_start(out=ident_sb[:], in_=ident_in[:])
            d_sb = const_pool.tile([P, DCOLS], dt.bfloat16)
            nc.sync.dma_start(out=d_sb[:], in_=d_in[:])

            for si in range(NSB):
                icols = int(idx_cols[si])
                ioff = int(idx_off[si])
                idx_sb = idx_pool.tile([P, IDX_MAX], dt.int16)
                nc.sync.dma_start(out=idx_sb[:, :icols],
                                  in_=idx_in[:, ioff:ioff + icols])

                msg = msg_pool.tile([P, MSG_MAX, P], dt.bfloat16)
                nc.sync.dma_start(
                    out=msg[:, 0:SBW, :],
                    in_=self_in[:, si * SBW * P:(si + 1) * SBW * P]
                        .rearrange("p (b f) -> p b f", b=SBW))
                for ki in range(NK):
                    n_idx = int(sec_pad[si, ki])
                    mcol0 = int(sec_col0[si, ki])
                    icol0 = int(idx_col0[si, ki])
                    o0 = 0
                    while o0 < n_idx:
                        nn = min(max_call_idx, n_idx - o0)
                        nc.gpsimd.dma_gather(
                            msg[:, mcol0 + o0 // P: mcol0 + (o0 + nn) // P, :],
                            src_tab[ki * cfg.src_window:
                                    (ki + 1) * cfg.src_window, :],
                            idx_sb[:, icol0 + o0 // 16:
                                   icol0 + (o0 + nn) // 16],
                            nn, nn, P,
                            queue_num=ki % 4,
                            single_packet=single_packet,
                        )
                        o0 += nn

                out_sb = out_pool.tile([P, SBW * out_f], out_dt)
                for b in range(SBW):
                    b_loc = si * SBW + b
                    nd = int(ndcols[si, b])
                    dc0 = int(dcol_base[si, b])
                    sel = sel_pool.tile([P, G_MAX * P], dt.bfloat16)
                    nc.vector.tensor_tensor(
                        sel[:, :nd * P],
                        iota_sb[:, :nd * P],
                        d_sb[:, dc0:dc0 + nd].to_broadcast([P, nd, P]),
                        mybir.AluOpType.is_equal,
                    )

                    agg = agg_psum.tile([P, P], dt.float32)
                    ents = mm_sched[si][b]
                    for ei, (mcol, dci) in enumerate(ents):
                        rhs = (ident_sb[:] if dci < 0
                               else sel[:, dci * P:(dci + 1) * P])
                        nc.tensor.matmul(
                            agg[:],
                            lhsT=msg[:, mcol, :],
                            rhs=rhs,
                            start=(ei == 0), stop=(ei == len(ents) - 1),
                        )

                    aggsb = aggsb_pool.tile([P, P], dt.bfloat16, tag="aggsb")
                    nc.scalar.copy(aggsb[:], agg[:])

                    # z[d, fout] = aggsb^T @ W   (dest-major)
                    zps = z_psum.tile([P, out_f], dt.float32)
                    nc.tensor.matmul(zps[:], lhsT=aggsb[:], rhs=w_sb[:],
                                     start=True, stop=True)

                    dv = dinv_sb[:, b_loc:b_loc + 1]
                    o_sl = out_sb[:, b * out_f:(b + 1) * out_f]
                    if layer == 1:
                        # t = zps * dinv_d  (ACT per-partition scale, cast)
                        t = tmp_pool.tile([P, out_f], dt.bfloat16, tag="t")
                        nc.scalar.activation(
                            t[:], zps[:], mybir.ActivationFunctionType.Copy,
                            scale=dv)
                        # u = t + b  (DVE, bf16 2x)
                        u = tmp_pool.tile([P, out_f], dt.bfloat16, tag="u")
                        nc.vector.tensor_tensor(u[:], t[:], btile_sb[:],
                                                mybir.AluOpType.add)
                        # rr = max(slope*u, u)  (DVE)
                        rr = tmp_pool.tile([P, out_f], dt.bfloat16, tag="rr")
                        nc.vector.scalar_tensor_tensor(
                            rr[:], u[:], float(RRELU_SLOPE), u[:],
                            mybir.AluOpType.mult, mybir.AluOpType.max)
                        # gs = rr * dinv_d  (ACT per-partition scale)
                        nc.scalar.activation(
                            o_sl, rr[:], mybir.ActivationFunctionType.Copy,
                            scale=dv)
                    else:
                        t = tmp_pool.tile([P, out_f], dt.float32, tag="t")
                        nc.scalar.activation(
                            t[:], zps[:], mybir.ActivationFunctionType.Copy,
                            scale=dv)
                        nc.vector.tensor_tensor(o_sl, t[:], btile_sb[:],
                                                mybir.AluOpType.add)

                nc.sync.dma_start(
                    out=out_view[si],
                    in_=out_sb[:].rearrange("p (b f) -> p b f", b=SBW))

    nc.compile()
    return nc


# --------------------------------------------------------------------------
# orchestration
# --------------------------------------------------------------------------

def _iota_tile(G):
    return (np.tile(np.arange(P, dtype=np.float32), G)[None, :]
            .repeat(P, 0).astype(ml_dtypes.bfloat16))


def _self_tab(xs, cfg, c):
    v = xs[c * cfg.nodes_per_core:(c + 1) * cfg.nodes_per_core]
    v = v.reshape(cfg.bpc, P, P).transpose(1, 0, 2).reshape(P, cfg.bpc * P)
    return np.ascontiguousarray(v)


def _run_gcn(x, edge_index, W1, b1, W2, b2, cfg, runner=None, want_times=False):
    meta = preprocess(np.asarray(edge_index), cfg)
    dinv = meta["dinv"]
    shared = meta["shared"]
    npc = cfg.nodes_per_core

    if runner is None:
        times = []

        def runner(nc, in_maps):
            r = run_bass_kernel_spmd(nc, in_maps, core_ids=list(range(cfg.n_cores)),
                                     trace=want_times)
            if want_times:
                times.append(r.exec_time_ns)
            return r.results
    else:
        times = None

    x = np.asarray(x, dtype=np.float32)
    xs = np.zeros((cfg.tab_rows, P), dtype=ml_dtypes.bfloat16)
    xs[:cfg.n_nodes] = (x * dinv[:cfg.n_nodes, None]).astype(ml_dtypes.bfloat16)

    iota = _iota_tile(shared["G_MAX"])
    ident = np.eye(P, dtype=np.float32).astype(ml_dtypes.bfloat16)
    w1 = np.asarray(W1, np.float32).astype(ml_dtypes.bfloat16)
    w2 = np.asarray(W2, np.float32).astype(ml_dtypes.bfloat16)
    bt1 = np.tile(np.asarray(b1, np.float32)[None, :], (P, 1)).astype(ml_dtypes.bfloat16)
    bt2 = np.ascontiguousarray(np.tile(np.asarray(b2, np.float32)[None, :], (P, 1)))

    nc1 = build_layer_program(cfg, shared, layer=1,
                              max_call_idx=8064, single_packet=False)
    in_maps = [
        {"src_tab": xs, "w": w1, "btile": bt1, "iota": iota, "ident": ident,
         "self_tab": _self_tab(xs, cfg, c),
         **{kk: pc[kk] for kk in ("idx_tab", "d_tab", "dinv_pm")}}
        for c, pc in enumerate(meta["per_core"])
    ]
    res1 = runner(nc1, in_maps)

    gs = np.zeros((cfg.tab_rows, P), dtype=ml_dtypes.bfloat16)
    for c in range(cfg.n_cores):
        gs[c * npc:(c + 1) * npc] = res1[c]["out_t"]

    nc2 = build_layer_program(cfg, shared, layer=2,
                              max_call_idx=896, single_packet=True)
    for c in range(cfg.n_cores):
        in_maps[c] = dict(in_maps[c])
        in_maps[c]["src_tab"] = gs
        in_maps[c]["self_tab"] = _self_tab(gs, cfg, c)
        in_maps[c]["w"] = w2
        in_maps[c]["btile"] = bt2
    res2 = runner(nc2, in_maps)

    out = np.zeros((cfg.n_pad, cfg.out2_f), dtype=np.float32)
    for c in range(cfg.n_cores):
        out[c * npc:(c + 1) * npc] = res2[c]["out_t"]
    out = out[:cfg.n_nodes]
    if want_times and times is not None:
        return out, times
    return out


def kernel(x, edge_index, W1, b1, W2, b2):
    return _run_gcn(x, edge_index, W1, b1, W2, b2, FULL)


# revision 10
# speedup vs baseline: 1.7367x; 1.7367x over previous
"""Two-layer GCN (PyG GCNConv x2 + rrelu) on 8 Trainium2 NeuronCores.

Math: with A = adjacency-with-multiplicity + I (self loops), deg = in-degree
(including the self loop), dinv = deg^-1/2:
    z1[v] = dinv[v] * (sum_{u->v} dinv[u]*x[u]) @ W1 + b1
    g[u]  = dinv[u] * rrelu(z1[u])                      (dinv pre-folded for L2)
    z2[v] = dinv[v] * (sum_{u->v} g[u]) @ W2 + b2

Sharding: destinations range-sharded across 8 cores (12544 each).  Every core
keeps a replicated (dinv-prescaled, bf16) source-feature table in HBM and
fetches the source rows of its edges with big dma_gather calls (one per
(superblock, source-window), ~4K indices).  Edge slots are packed
back-to-back with per-(block,window) segment lengths fixed to the max across
cores (SPMD uniformity).  Self-loop rows are staged host-side in a
partition-major layout and land in the message tile via one fat DMA per
superblock; their selector is the constant identity.  Scatter onto
destinations is a TensorE matmul with one-hot selectors generated on DVE (one
is_equal per destination block).  The epilogue runs in destination-major
orientation [dest, feat] so the per-destination dinv factors apply via the
ScalarE activation per-partition scale operand; outputs are written
node-major, which doubles as the next layer's source-table layout.
"""

import sys

for _p in ("/opt/trn_rl_repo",):
    if _p not in sys.path:
        sys.path.insert(0, _p)

import numpy as np
import ml_dtypes

import concourse.bacc as bacc
import concourse.bass as bass
import concourse.mybir as mybir
import concourse.tile as tile
from concourse.bass_utils import run_bass_kernel_spmd

P = 128
RRELU_SLOPE = (1.0 / 8.0 + 1.0 / 3.0) / 2.0


class Cfg:
    def __init__(self, n_nodes, n_cores, blocks_per_core, superblock, in_f,
                 out1_f, out2_f, src_window):
        self.n_nodes = n_nodes
        self.n_cores = n_cores
        self.bpc = blocks_per_core
        self.sb = superblock
        assert blocks_per_core % superblock == 0
        self.sb_count = blocks_per_core // superblock
        self.in_f = in_f
        self.out1_f = out1_f
        self.out2_f = out2_f
        self.src_window = src_window
        self.nodes_per_core = blocks_per_core * P
        self.n_pad = n_cores * self.nodes_per_core
        assert self.n_pad >= n_nodes
        self.n_chunks = -(-self.n_pad // src_window)
        self.tab_rows = self.n_chunks * src_window


FULL = Cfg(n_nodes=100000, n_cores=8, blocks_per_core=98, superblock=7,
           in_f=128, out1_f=128, out2_f=64, src_window=25088)


def _ru(x, m):
    return -(-x // m) * m


# --------------------------------------------------------------------------
# host-side index preprocessing
# --------------------------------------------------------------------------

def preprocess(edge_index, cfg):
    row = edge_index[0].astype(np.int64)
    col = edge_index[1].astype(np.int64)
    n = cfg.n_nodes
    npc = cfg.nodes_per_core
    NSB, NK, SBW = cfg.sb_count, cfg.n_chunks, cfg.sb

    deg = np.bincount(col, minlength=cfg.n_pad).astype(np.float64) + 1.0
    dinv = (1.0 / np.sqrt(deg)).astype(np.float32)
    dinv[n:] = 1.0

    core = col // npc
    col_loc = col % npc
    blk = col_loc >> 7
    s = blk // SBW
    b7 = blk % SBW
    k = row // cfg.src_window
    dloc = col_loc & 127

    cnt = np.zeros((cfg.n_cores, NSB, NK, SBW), dtype=np.int64)
    np.add.at(cnt, (core, s, k, b7), 1)
    seg_len = cnt.max(axis=0)             # [NSB, NK, SBW] uniform

    seg_start = np.zeros_like(seg_len)
    sec_pad = np.zeros((NSB, NK), dtype=np.int64)
    for si in range(NSB):
        for ki in range(NK):
            c0 = 0
            for b in range(SBW):
                seg_start[si, ki, b] = c0
                c0 += seg_len[si, ki, b]
            sec_pad[si, ki] = _ru(max(c0, 1), P)

    # msg tile columns: cols 0..6 = self rows; then per-k sections
    sec_col0 = np.zeros((NSB, NK), dtype=np.int64)
    msg_cols = np.zeros(NSB, dtype=np.int64)
    for si in range(NSB):
        c = SBW
        for ki in range(NK):
            sec_col0[si, ki] = c
            c += sec_pad[si, ki] // P
        msg_cols[si] = c

    idx_col0 = np.zeros((NSB, NK), dtype=np.int64)
    idx_cols = np.zeros(NSB, dtype=np.int64)
    for si in range(NSB):
        c = 0
        for ki in range(NK):
            idx_col0[si, ki] = c
            c += sec_pad[si, ki] // 16
        idx_cols[si] = c
    idx_off = np.concatenate([[0], np.cumsum(idx_cols)])
    ICOLS = int(idx_off[-1])

    # matmul schedule: per (s, b7) list of (msg_col, dcol) with dcol=-1 for
    # the identity (self) column
    mm_sched = [[None] * SBW for _ in range(NSB)]
    ndcols = np.zeros((NSB, SBW), dtype=np.int64)
    dcol_of = {}
    dcol_base = np.zeros((NSB, SBW), dtype=np.int64)
    DCOLS = 0
    for si in range(NSB):
        for b in range(SBW):
            ents = [(b, -1)]
            nd = 0
            for ki in range(NK):
                st = int(seg_start[si, ki, b])
                ln = int(seg_len[si, ki, b])
                if ln == 0:
                    continue
                c0, c1 = st // P, (st + ln - 1) // P
                for cc in range(c0, c1 + 1):
                    mcol = int(sec_col0[si, ki]) + cc
                    ents.append((mcol, nd))
                    dcol_of[(si, ki, cc, b)] = nd
                    nd += 1
            mm_sched[si][b] = ents
            ndcols[si, b] = nd
            dcol_base[si, b] = DCOLS
            DCOLS += nd
    G_MAX = int(ndcols.max())

    # per-edge slot assignment
    gid = ((core * NSB + s) * NK + k) * SBW + b7
    order = np.argsort(gid, kind="stable")
    gsort = gid[order]
    grp_start = np.zeros(cfg.n_cores * NSB * NK * SBW + 1, dtype=np.int64)
    np.cumsum(np.bincount(gsort, minlength=grp_start.size - 1), out=grp_start[1:])
    rank = np.empty(row.size, dtype=np.int64)
    rank[order] = np.arange(row.size) - grp_start[gsort]

    slot_in_sec = seg_start[s, k, b7] + rank
    sec_colv = slot_in_sec >> 7
    sec_p = slot_in_sec & 127
    idx_val = (row - k * cfg.src_window).astype(np.int16)
    idx_colv = idx_off[s] + idx_col0[s, k] + (slot_in_sec >> 4)
    idx_rowv = slot_in_sec & 15

    max_cols = int((sec_pad // P).max())
    dlk = np.full((NSB, NK, max_cols, SBW), -1, dtype=np.int64)
    for (si, ki, cc, b), v in dcol_of.items():
        dlk[si, ki, cc, b] = v
    dcol_l = dlk[s, k, sec_colv, b7]
    assert (dcol_l >= 0).all()
    d_colv = dcol_base[s, b7] + dcol_l

    per_core = []
    for c in range(cfg.n_cores):
        m = core == c
        it = np.zeros((16, ICOLS), dtype=np.int16)
        it[idx_rowv[m], idx_colv[m]] = idx_val[m]
        idx_tab = np.tile(it, (8, 1))
        d_tab = np.full((P, DCOLS), -1.0, dtype=np.float64)
        d_tab[sec_p[m], d_colv[m]] = dloc[m].astype(np.float64)
        dinv_pm = dinv[c * npc:(c + 1) * npc].reshape(cfg.bpc, P).T
        per_core.append({
            "idx_tab": np.ascontiguousarray(idx_tab),
            "d_tab": np.ascontiguousarray(d_tab.astype(ml_dtypes.bfloat16)),
            "dinv_pm": np.ascontiguousarray(dinv_pm),
        })

    shared = {
        "sec_pad": sec_pad, "sec_col0": sec_col0, "msg_cols": msg_cols,
        "idx_col0": idx_col0, "idx_cols": idx_cols, "idx_off": idx_off,
        "ICOLS": ICOLS, "DCOLS": DCOLS, "G_MAX": G_MAX,
        "mm_sched": mm_sched, "ndcols": ndcols, "dcol_base": dcol_base,
    }
    return {"dinv": dinv, "per_core": per_core, "shared": shared}


# --------------------------------------------------------------------------
# bass program (one GCN layer, SPMD across cores)
# --------------------------------------------------------------------------

def build_layer_program(cfg, shared, layer, max_call_idx=8064,
                        single_packet=False):
    NSB, NK, SBW = cfg.sb_count, cfg.n_chunks, cfg.sb
    out_f = cfg.out1_f if layer == 1 else cfg.out2_f
    out_dt = mybir.dt.bfloat16 if layer == 1 else mybir.dt.float32
    ICOLS, DCOLS, G_MAX = shared["ICOLS"], shared["DCOLS"], shared["G_MAX"]
    sec_pad, sec_col0 = shared["sec_pad"], shared["sec_col0"]
    msg_cols = shared["msg_cols"]
    idx_col0, idx_cols, idx_off = (shared["idx_col0"], shared["idx_cols"],
                                   shared["idx_off"])
    mm_sched, ndcols, dcol_base = (shared["mm_sched"], shared["ndcols"],
                                   shared["dcol_base"])
    MSG_MAX = int(msg_cols.max())
    IDX_MAX = int(idx_cols.max())

    nc = bacc.Bacc("TRN2", target_bir_lowering=False, debug=False,
                   num_devices=cfg.n_cores, num_swdge_queues=4)
    dt = mybir.dt
    src_tab = nc.dram_tensor("src_tab", [cfg.tab_rows, P], dt.bfloat16,
                             kind="ExternalInput")
    w_in = nc.dram_tensor("w", [P, out_f], dt.bfloat16, kind="ExternalInput")
    btile_in = nc.dram_tensor("btile", [P, out_f],
                              dt.bfloat16 if layer == 1 else dt.float32,
                              kind="ExternalInput")
    dinv_in = nc.dram_tensor("dinv_pm", [P, cfg.bpc], dt.float32,
                             kind="ExternalInput")
    idx_in = nc.dram_tensor("idx_tab", [P, ICOLS], dt.int16,
                            kind="ExternalInput")
    d_in = nc.dram_tensor("d_tab", [P, DCOLS], dt.bfloat16, kind="ExternalInput")
    iota_in = nc.dram_tensor("iota", [P, G_MAX * P], dt.bfloat16,
                             kind="ExternalInput")
    ident_in = nc.dram_tensor("ident", [P, P], dt.bfloat16, kind="ExternalInput")
    self_in = nc.dram_tensor("self_tab", [P, cfg.bpc * P], dt.bfloat16,
                             kind="ExternalInput")
    # partition-major output: out_t[p, blk*out_f + f] = out[blk*128 + p, f]
    out_t = nc.dram_tensor("out_t", [P, cfg.bpc * out_f], out_dt,
                           kind="ExternalOutput")

    with tile.TileContext(nc) as tc:
        with (
            tc.tile_pool(name="const", bufs=1) as const_pool,
            tc.tile_pool(name="idx", bufs=3) as idx_pool,
            tc.tile_pool(name="msg", bufs=3) as msg_pool,
            tc.tile_pool(name="sel", bufs=3) as sel_pool,
            tc.tile_pool(name="aggsb", bufs=3) as aggsb_pool,
            tc.tile_pool(name="tmp", bufs=3) as tmp_pool,
            tc.tile_pool(name="outsb", bufs=2) as out_pool,
            tc.tile_pool(name="psA", bufs=2, space="PSUM") as agg_psum,
            tc.tile_pool(name="psZ", bufs=2, space="PSUM") as z_psum,
        ):
            w_sb = const_pool.tile([P, out_f], dt.bfloat16)
            nc.sync.dma_start(out=w_sb[:], in_=w_in[:])
            btile_sb = const_pool.tile([P, out_f],
                                       dt.bfloat16 if layer == 1 else dt.float32)
            nc.sync.dma_start(out=btile_sb[:], in_=btile_in[:])
            dinv_sb = const_pool.tile([P, cfg.bpc], dt.float32)
            nc.sync.dma_start(out=dinv_sb[:], in_=dinv_in[:])
            iota_sb = const_pool.tile([P, G_MAX * P], dt.bfloat16)
            nc.sync.dma_start(out=iota_sb[:], in_=iota_in[:])
            ident_sb = const_pool.tile([P, P], dt.bfloat16)
            nc.sync.dma_start(out=ident_sb[:], in_=ident_in[:])
            d_sb = const_pool.tile([P, DCOLS], dt.bfloat16)
            nc.sync.dma_start(out=d_sb[:], in_=d_in[:])

            for si in range(NSB):
                icols = int(idx_cols[si])
                ioff = int(idx_off[si])
                idx_sb = idx_pool.tile([P, IDX_MAX], dt.int16)
                nc.sync.dma_start(out=idx_sb[:, :icols],
                                  in_=idx_in[:, ioff:ioff + icols])

                msg = msg_pool.tile([P, MSG_MAX, P], dt.bfloat16)
                nc.sync.dma_start(
                    out=msg[:, 0:SBW, :],
                    in_=self_in[:, si * SBW * P:(si + 1) * SBW * P]
                        .rearrange("p (b f) -> p b f", b=SBW))
                for ki in range(NK):
                    n_idx = int(sec_pad[si, ki])
                    mcol0 = int(sec_col0[si, ki])
                    icol0 = int(idx_col0[si, ki])
                    o0 = 0
                    while o0 < n_idx:
                        nn = min(max_call_idx, n_idx - o0)
                        nc.gpsimd.dma_gather(
                            msg[:, mcol0 + o0 // P: mcol0 + (o0 + nn) // P, :],
                            src_tab[ki * cfg.src_window:
                                    (ki + 1) * cfg.src_window, :],
                            idx_sb[:, icol0 + o0 // 16:
                                   icol0 + (o0 + nn) // 16],
                            nn, nn, P,
                            queue_num=ki % 4,
                            single_packet=single_packet,
                        )
                        o0 += nn

                out_sb = out_pool.tile([P, SBW * out_f], out_dt)
                for b in range(SBW):
                    b_loc = si * SBW + b
                    nd = int(ndcols[si, b])
                    dc0 = int(dcol_base[si, b])
                    sel = sel_pool.tile([P, G_MAX * P], dt.bfloat16)
                    nc.vector.tensor_tensor(
                        sel[:, :nd * P],
                        iota_sb[:, :nd * P],
                        d_sb[:, dc0:dc0 + nd].to_broadcast([P, nd, P]),
                        mybir.AluOpType.is_equal,
                    )

                    agg = agg_psum.tile([P, P], dt.float32)
                    ents = mm_sched[si][b]
                    for ei, (mcol, dci) in enumerate(ents):
                        rhs = (ident_sb[:] if dci < 0
                               else sel[:, dci * P:(dci + 1) * P])
                        nc.tensor.matmul(
                            agg[:],
                            lhsT=msg[:, mcol, :],
                            rhs=rhs,
                            start=(ei == 0), stop=(ei == len(ents) - 1),
                        )

                    aggsb = aggsb_pool.tile([P, P], dt.bfloat16, tag="aggsb")
                    nc.scalar.copy(aggsb[:], agg[:])

                    # z[d, fout] = aggsb^T @ W   (dest-major)
                    zps = z_psum.tile([P, out_f], dt.float32)
                    nc.tensor.matmul(zps[:], lhsT=aggsb[:], rhs=w_sb[:],
                                     start=True, stop=True)

                    dv = dinv_sb[:, b_loc:b_loc + 1]
                    o_sl = out_sb[:, b * out_f:(b + 1) * out_f]
                    if layer == 1:
                        # t = zps * dinv_d  (ACT per-partition scale, cast)
                        t = tmp_pool.tile([P, out_f], dt.bfloat16, tag="t")
                        nc.scalar.activation(
                            t[:], zps[:], mybir.ActivationFunctionType.Copy,
                            scale=dv)
                        # u = t + b  (DVE, bf16 2x)
                        u = tmp_pool.tile([P, out_f], dt.bfloat16, tag="u")
                        nc.vector.tensor_tensor(u[:], t[:], btile_sb[:],
                                                mybir.AluOpType.add)
                        # rr = max(slope*u, u)  (DVE)
                        rr = tmp_pool.tile([P, out_f], dt.bfloat16, tag="rr")
                        nc.vector.scalar_tensor_tensor(
                            rr[:], u[:], float(RRELU_SLOPE), u[:],
                            mybir.AluOpType.mult, mybir.AluOpType.max)
                        # gs = rr * dinv_d  (ACT per-partition scale)
                        nc.scalar.activation(
                            o_sl, rr[:], mybir.ActivationFunctionType.Copy,
                            scale=dv)
                    else:
                        t = tmp_pool.tile([P, out_f], dt.float32, tag="t")
                        nc.scalar.activation(
                            t[:], zps[:], mybir.ActivationFunctionType.Copy,
                            scale=dv)
                        nc.vector.tensor_tensor(o_sl, t[:], btile_sb[:],
                                                mybir.AluOpType.add)

                nc.sync.dma_start(
                    out=out_t[:, si * SBW * out_f:(si + 1) * SBW * out_f],
                    in_=out_sb[:])

    nc.compile()
    return nc


# --------------------------------------------------------------------------
# orchestration
# --------------------------------------------------------------------------

def _iota_tile(G):
    return (np.tile(np.arange(P, dtype=np.float32), G)[None, :]
            .repeat(P, 0).astype(ml_dtypes.bfloat16))


def _self_tab(xs, cfg, c):
    v = xs[c * cfg.nodes_per_core:(c + 1) * cfg.nodes_per_core]
    v = v.reshape(cfg.bpc, P, P).transpose(1, 0, 2).reshape(P, cfg.bpc * P)
    return np.ascontiguousarray(v)


def _run_gcn(x, edge_index, W1, b1, W2, b2, cfg, runner=None, want_times=False):
    meta = preprocess(np.asarray(edge_index), cfg)
    dinv = meta["dinv"]
    shared = meta["shared"]
    npc = cfg.nodes_per_core

    if runner is None:
        times = []

        def runner(nc, in_maps):
            r = run_bass_kernel_spmd(nc, in_maps, core_ids=list(range(cfg.n_cores)),
                                     trace=want_times)
            if want_times:
                times.append(r.exec_time_ns)
            return r.results
    else:
        times = None

    x = np.asarray(x, dtype=np.float32)
    xs = np.zeros((cfg.tab_rows, P), dtype=ml_dtypes.bfloat16)
    xs[:cfg.n_nodes] = (x * dinv[:cfg.n_nodes, None]).astype(ml_dtypes.bfloat16)

    iota = _iota_tile(shared["G_MAX"])
    ident = np.eye(P, dtype=np.float32).astype(ml_dtypes.bfloat16)
    w1 = np.asarray(W1, np.float32).astype(ml_dtypes.bfloat16)
    w2 = np.asarray(W2, np.float32).astype(ml_dtypes.bfloat16)
    bt1 = np.tile(np.asarray(b1, np.float32)[None, :], (P, 1)).astype(ml_dtypes.bfloat16)
    bt2 = np.ascontiguousarray(np.tile(np.asarray(b2, np.float32)[None, :], (P, 1)))

    nc1 = build_layer_program(cfg, shared, layer=1,
                              max_call_idx=8064, single_packet=False)
    in_maps = [
        {"src_tab": xs, "w": w1, "btile": bt1, "iota": iota, "ident": ident,
         "self_tab": _self_tab(xs, cfg, c),
         **{kk: pc[kk] for kk in ("idx_tab", "d_tab", "dinv_pm")}}
        for c, pc in enumerate(meta["per_core"])
    ]
    res1 = runner(nc1, in_maps)

    gs = np.zeros((cfg.tab_rows, P), dtype=ml_dtypes.bfloat16)
    for c in range(cfg.n_cores):
        o = res1[c]["out_t"].reshape(P, cfg.bpc, cfg.out1_f)
        gs[c * npc:(c + 1) * npc] = o.transpose(1, 0, 2).reshape(npc, cfg.out1_f)

    nc2 = build_layer_program(cfg, shared, layer=2,
                              max_call_idx=8064, single_packet=False)
    for c in range(cfg.n_cores):
        in_maps[c] = dict(in_maps[c])
        in_maps[c]["src_tab"] = gs
        in_maps[c]["self_tab"] = _self_tab(gs, cfg, c)
        in_maps[c]["w"] = w2
        in_maps[c]["btile"] = bt2
    res2 = runner(nc2, in_maps)

    out = np.zeros((cfg.n_pad, cfg.out2_f), dtype=np.float32)
    for c in range(cfg.n_cores):
        o = res2[c]["out_t"].reshape(P, cfg.bpc, cfg.out2_f)
        out[c * npc:(c + 1) * npc] = o.transpose(1, 0, 2).reshape(npc, cfg.out2_f)
    out = out[:cfg.n_nodes]
    if want_times and times is not None:
        return out, times
    return out


def kernel(x, edge_index, W1, b1, W2, b2):
    return _run_gcn(x, edge_index, W1, b1, W2, b2, FULL)
